# revision 1
# baseline (speedup 1.0000x reference)
"""SpMM (GCN layer) kernel for 8 TRN2 NeuronCores.

out[i] = sum_{e: row[e]==i} vals[e] * embeds[col[e]]     (N=100000, E=3.2M, d=32)

Strategy (1D row partition, per the classic SpMM sharding):
  - Host sorts edges by destination row and shards rows 8 ways
    (12500 rows / ~400K edges per core).  The embedding table is
    replicated to every core in HBM.
  - Per core the edges are laid out in a padded-CSR "slot" structure:
    each (row, col-chunk) gets K4 slots; slot j lives at SBUF partition
    j%128, free column j//128, so each row's slots are contiguous in one
    partition.  A hardware batched gather (InstDMAGatherAnt, 128B
    payload / 256B stride descriptors) fetches embeds[col] for every
    slot; pad slots point at an all-zero table row.
  - DVE scales by vals (broadcast multiply) and does a static
    segmented reduction (tensor_reduce over the K4 axis), accumulating
    the 4 col-chunks into a resident [128, 98, 32] output tile.
  - Rows with more than K4 edges in a chunk spill to small per-128-row
    "overflow" batches handled with indirect gather + CCE-add scatter.
  - Dense output write, host concatenates the 8 row blocks.
"""

import sys

if "/opt/trn_rl_repo" not in sys.path:
    sys.path.insert(0, "/opt/trn_rl_repo")

import numpy as np

import concourse.bass as bass
import concourse.tile as tile
from concourse import bacc, mybir, bass_utils
from concourse import ap_utils
from concourse.bass import round_up_to_multiple, exact_div
from concourse.tile import add_dep_helper

# ---------------- problem geometry (hardcoded) ----------------
N_NODES = 100000
N_EDGES = 3200000
D = 32
NC = 8
RPC = N_NODES // NC            # rows per core = 12500
RPAD = 12544                   # rows padded to multiple of 128 (98 * 128)
QROWS = RPAD // 128            # 98 free "q" rows per partition
CHUNK = 25000                  # nodes per table chunk (balanced, int16-safe)
NCHUNKS = 4
K4 = 12                        # slots per (row, chunk)
COLS = QROWS * K4              # free columns per chunk's slot block = 1176
SLOTS = COLS * 128             # slots per chunk = 150528
TCOL = 24                      # free columns per gather instruction
NBLK = COLS // TCOL            # 49 gather instructions per chunk
NIDX = TCOL * 128              # 3072 indices per gather instruction
ELEM = 32                      # gathered payload elems (128B)
STEP = 64                      # table row stride elems (256B)
PAD_LIDX = CHUNK               # local index of the zero row in each chunk
DEAD_ROW = RPC + 10            # scatter target for overflow padding lanes

_cache = {}


def _dma_gather_raw(gp, out_ap, in_ap, idxs_ap, num_idxs, num_idxs_reg,
                    elem_size, elem_step, queue_num=0):
    """dma_gather with a 128-byte payload on a 256-byte-strided table.

    Identical to BassGpSimd.dma_gather(transpose=False) minus the
    blanket elem_size_bytes%256 assert (only the descriptor stride needs
    the 256B granularity; payload size is free).  Validated on HW.
    """
    assert idxs_ap.dtype == mybir.dt.int16
    assert in_ap.dtype == out_ap.dtype
    assert in_ap.space == bass.MemorySpace.DRAM
    assert idxs_ap.space == bass.MemorySpace.SBUF
    assert out_ap.space == bass.MemorySpace.SBUF
    assert ap_utils.ap_is_contiguous(in_ap.ap[1:])
    assert ap_utils.ap_is_contiguous(out_ap.ap[1:])
    assert ap_utils.ap_is_contiguous(idxs_ap.ap[1:])
    assert in_ap.ap[-1][1] == out_ap.ap[-1][1] == elem_size
    assert out_ap.ap[0][1] * out_ap.ap[1][1] == round_up_to_multiple(num_idxs, 128)
    assert in_ap.ap[0][0] == elem_step
    stride_bytes = elem_step * mybir.dt.size(in_ap.dtype)
    stride_bytes_256 = exact_div(stride_bytes, 256)
    assert stride_bytes_256 < 256
    _in_ap = gp.lower_ap_dma(in_ap, for_custom_bir_dma=True)
    _idxs_ap = gp.lower_ap(idxs_ap)
    _out_ap = gp.lower_ap(out_ap)
    return gp.add_instruction(
        mybir.InstDMAGatherAnt(
            name=gp.bass.get_next_instruction_name(),
            ins=[*_in_ap, _idxs_ap, gp.lower_val_access(gp.to_reg(num_idxs_reg))],
            outs=[_out_ap],
            transpose=False,
            num_idxs=num_idxs,
            elem_size=elem_size,
            stride_bytes_256=stride_bytes_256,
            gen_mode=0,
            single_packet=False,
            queue_num=queue_num,
            sbuf_tokens_per_rank=0,
            sbuf_free_dim_per_rank=0,
            sbuf_free_dim_pad_per_rank=0,
            sbuf_byte_offset=0,
        )
    )


def _build_module(novi):
    """Build + finalize the per-core SPMD module. novi = number of
    128-lane overflow batches (identical across cores)."""
    nc = bacc.Bacc("TRN2", target_bir_lowering=False, num_swdge_queues=2)
    f32, i16, i32 = mybir.dt.float32, mybir.dt.int16, mybir.dt.int32

    tabp = nc.dram_tensor("tabp", [NCHUNKS * 32768, STEP], f32, kind="ExternalInput")
    emb = nc.dram_tensor("emb", [N_NODES, D], f32, kind="ExternalInput")
    idxs = nc.dram_tensor("idxs", [NCHUNKS, 128, SLOTS // 16], i16, kind="ExternalInput")
    vals = nc.dram_tensor("vals", [NCHUNKS, 128, COLS], f32, kind="ExternalInput")
    if novi:
        oidx = nc.dram_tensor("oidx", [novi, 128, 1], i32, kind="ExternalInput")
        oval = nc.dram_tensor("oval", [novi, 128, 1], f32, kind="ExternalInput")
        orow = nc.dram_tensor("orow", [novi, 128, 1], i32, kind="ExternalInput")
    y = nc.dram_tensor("y", [RPAD, D], f32, kind="ExternalOutput")

    with tile.TileContext(nc) as tc:
        with tc.tile_pool(name="acc", bufs=1) as accp, \
             tc.tile_pool(name="work", bufs=6) as wp:
            out_acc = accp.tile([128, QROWS * D], f32)
            nc.vector.memset(out_acc[:], 0.0)

            for c in range(NCHUNKS):
                tab_c = tabp[c * 32768:(c + 1) * 32768, :ELEM]
                for b in range(NBLK):
                    idx_t = wp.tile([128, TCOL * 8], i16, tag="idx")
                    val_t = wp.tile([128, TCOL], f32, tag="val")
                    g_t = wp.tile([128, TCOL, ELEM], f32, tag="g")
                    red_t = wp.tile([128, (TCOL // K4) * D], f32, tag="red")
                    nc.sync.dma_start(
                        out=idx_t[:],
                        in_=idxs[c, :, b * TCOL * 8:(b + 1) * TCOL * 8])
                    nc.scalar.dma_start(
                        out=val_t[:],
                        in_=vals[c, :, b * TCOL:(b + 1) * TCOL])
                    _dma_gather_raw(
                        nc.gpsimd, g_t[:], tab_c, idx_t[:],
                        num_idxs=NIDX, num_idxs_reg=NIDX,
                        elem_size=ELEM, elem_step=STEP,
                        queue_num=b % 2)
                    # scale by vals (broadcast along d)
                    vb = val_t[:].unsqueeze(-1).broadcast_to((128, TCOL, ELEM))
                    nc.vector.tensor_tensor(
                        out=g_t[:], in0=g_t[:], in1=vb,
                        op=mybir.AluOpType.mult)
                    # segmented reduce over the K4 axis (innermost, strided)
                    rin = g_t[:].rearrange("p (q k) d -> p q d k", k=K4)
                    nc.vector.tensor_reduce(
                        out=red_t[:].rearrange("p (q d) -> p q d", d=D),
                        in_=rin, axis=mybir.AxisListType.X,
                        op=mybir.AluOpType.add)
                    q0 = b * (TCOL // K4)
                    q1 = q0 + TCOL // K4
                    nc.vector.tensor_tensor(
                        out=out_acc[:, q0 * D:q1 * D],
                        in0=out_acc[:, q0 * D:q1 * D],
                        in1=red_t[:],
                        op=mybir.AluOpType.add)

            dense_wr = nc.sync.dma_start(
                out=y[:, :].rearrange("(q p) d -> p q d", p=128),
                in_=out_acc[:].rearrange("p (q d) -> p q d", d=D))

            prev = dense_wr
            for j in range(novi):
                oi_t = wp.tile([128, 1], i32, tag="oi")
                ov_t = wp.tile([128, 1], f32, tag="ov")
                or_t = wp.tile([128, 1], i32, tag="or")
                og_t = wp.tile([128, D], f32, tag="og")
                nc.sync.dma_start(out=oi_t[:], in_=oidx[j, :, :])
                nc.scalar.dma_start(out=ov_t[:], in_=oval[j, :, :])
                nc.sync.dma_start(out=or_t[:], in_=orow[j, :, :])
                nc.gpsimd.indirect_dma_start(
                    out=og_t[:], out_offset=None,
                    in_=emb[:, :],
                    in_offset=bass.IndirectOffsetOnAxis(ap=oi_t[:, :1], axis=0))
                nc.vector.tensor_scalar(
                    out=og_t[:], in0=og_t[:],
                    scalar1=ov_t[:, :1], scalar2=None,
                    op0=mybir.AluOpType.mult)
                sc = nc.gpsimd.indirect_dma_start(
                    out=y[:, :],
                    out_offset=bass.IndirectOffsetOnAxis(ap=or_t[:, :1], axis=0),
                    in_=og_t[:], in_offset=None,
                    compute_op=mybir.AluOpType.add)
                # CCE-add RMWs on y must run after the dense write and
                # serialize among themselves (rows can repeat across
                # batches); Tile's shadow-memory tracking orders same-
                # tensor DRAM accesses, verified by the row check in test.py
                prev = sc

    nc.finalize()
    return nc


def _preprocess(adj_row, adj_col, adj_vals, embeds):
    """Host-side shard + layout. Returns (in_maps, novi)."""
    order = np.argsort(adj_row, kind="stable")
    rows = np.ascontiguousarray(adj_row[order])
    cols = np.ascontiguousarray(adj_col[order])
    vals = np.ascontiguousarray(adj_vals[order]).astype(np.float32)

    # replicated padded table: chunk c rows [c*32768, c*32768+32767) hold
    # nodes [c*CHUNK, c*CHUNK+32767); row c*32768+32767 stays zero (pad)
    tabp = np.zeros((NCHUNKS * 32768, STEP), dtype=np.float32)
    for c in range(NCHUNKS):
        lo = c * CHUNK
        hi = min(lo + CHUNK, N_NODES)
        if lo < N_NODES:
            tabp[c * 32768:c * 32768 + (hi - lo), :D] = embeds[lo:hi]

    bounds = np.searchsorted(rows, np.arange(NC + 1) * RPC)

    per_core = []
    max_novi = 0
    for m in range(NC):
        s, e = bounds[m], bounds[m + 1]
        rl = (rows[s:e] - m * RPC).astype(np.int64)
        cc = cols[s:e].astype(np.int64)
        vv = vals[s:e]
        ch = cc // CHUNK
        lidx = cc % CHUNK

        # pad slots carry val=0, so they may gather ANY row (junk*0 == 0).
        # Spread them over the chunk so they don't hammer one HBM row
        # (a constant index measured 2.5x slower than random access).
        rng = np.random.default_rng(12345 + m)
        idx_arr = rng.integers(0, CHUNK, size=(NCHUNKS, SLOTS)).astype(np.int16)
        val_arr = np.zeros((NCHUNKS, SLOTS), dtype=np.float32)
        over = []  # (global_col, val, row_local, excess_rank)
        for c in range(NCHUNKS):
            msk = ch == c
            crl = rl[msk]          # sorted (rows sorted, mask keeps order)
            cli = lidx[msk]
            cvv = vv[msk]
            if crl.size == 0:
                continue
            uniq, starts, counts = np.unique(
                crl, return_index=True, return_counts=True)
            k = np.arange(crl.size) - np.repeat(starts, counts)
            main = k < K4
            mrl, mk = crl[main], k[main]
            slot = ((mrl // 128) * K4 + mk) * 128 + (mrl % 128)
            idx_arr[c, slot] = cli[main].astype(np.int16)
            val_arr[c, slot] = cvv[main]
            ov = ~main
            if ov.any():
                over.append((cc[msk][ov], cvv[ov], crl[ov], k[ov] - K4))

        # wrapped idx layout [128, SLOTS//16] (j%16 partition, replicated x8)
        idx_w = np.tile(
            idx_arr.reshape(NCHUNKS, SLOTS // 16, 16).transpose(0, 2, 1),
            (1, 8, 1))
        val_w = val_arr.reshape(NCHUNKS, COLS, 128).transpose(0, 2, 1)
        val_w = np.ascontiguousarray(val_w)

        # overflow batches: excess-rank groups chunked into 128-lane
        # batches; rows within a batch are distinct by construction
        ob = []
        if over:
            ocol = np.concatenate([o[0] for o in over])
            ovv = np.concatenate([o[1] for o in over])
            orl = np.concatenate([o[2] for o in over])
            # recompute excess rank ACROSS chunks per row for distinctness
            o_order = np.argsort(orl, kind="stable")
            ocol, ovv, orl = ocol[o_order], ovv[o_order], orl[o_order]
            uniq, starts, counts = np.unique(
                orl, return_index=True, return_counts=True)
            be = np.arange(orl.size) - np.repeat(starts, counts)
            for b in range(int(be.max()) + 1):
                sel = be == b
                sc_, sv_, sr_ = ocol[sel], ovv[sel], orl[sel]
                for i in range(0, sc_.size, 128):
                    ci = np.zeros(128, np.int32)
                    cv = np.zeros(128, np.float32)
                    cr = np.full(128, DEAD_ROW, np.int32)
                    n = min(128, sc_.size - i)
                    ci[:n] = sc_[i:i + n]
                    cv[:n] = sv_[i:i + n]
                    cr[:n] = sr_[i:i + n]
                    ob.append((ci, cv, cr))
        max_novi = max(max_novi, len(ob))
        per_core.append((idx_w, val_w, ob))

    novi = max_novi
    in_maps = []
    for m in range(NC):
        idx_w, val_w, ob = per_core[m]
        im = {
            "tabp": tabp,
            "emb": np.ascontiguousarray(embeds.astype(np.float32)),
            "idxs": np.ascontiguousarray(idx_w),
            "vals": val_w,
        }
        if novi:
            oidx = np.zeros((novi, 128, 1), np.int32)
            oval = np.zeros((novi, 128, 1), np.float32)
            orow = np.full((novi, 128, 1), DEAD_ROW, np.int32)
            for j, (ci, cv, cr) in enumerate(ob):
                oidx[j, :, 0] = ci
                oval[j, :, 0] = cv
                orow[j, :, 0] = cr
            im["oidx"], im["oval"], im["orow"] = oidx, oval, orow
        in_maps.append(im)
    return in_maps, novi


def _run(in_maps, novi, trace=False):
    key = ("mod", novi)
    if key not in _cache:
        _cache[key] = _build_module(novi)
    nc = _cache[key]
    return bass_utils.run_bass_kernel_spmd(
        nc, in_maps, core_ids=list(range(NC)), trace=trace)


def kernel(adj_row, adj_col, adj_vals, embeds, _trace=False, _return_res=False):
    adj_row = np.asarray(adj_row)
    adj_col = np.asarray(adj_col)
    adj_vals = np.asarray(adj_vals)
    embeds = np.asarray(embeds)
    in_maps, novi = _preprocess(adj_row, adj_col, adj_vals, embeds)
    res = _run(in_maps, novi, trace=_trace)
    out = np.concatenate(
        [res.results[m]["y"][:RPC] for m in range(NC)], axis=0)
    out = np.ascontiguousarray(out, dtype=np.float32)
    if _return_res:
        return out, res
    return out



# revision 3
# speedup vs baseline: 3.7677x; 3.7677x over previous
"""SpMM (GCN layer) kernel for 8 TRN2 NeuronCores.

out[i] = sum_{e: row[e]==i} vals[e] * embeds[col[e]]     (N=100000, E=3.2M, d=32)

Strategy (1D row partition, per the classic SpMM sharding):
  - Host sorts edges by destination row and shards rows 8 ways
    (12500 rows / ~400K edges per core).  The embedding table is
    replicated to every core in HBM.
  - Per core the edges are laid out in a padded-CSR "slot" structure:
    each (row, col-chunk) gets K4 slots; slot j lives at SBUF partition
    j%128, free column j//128, so each row's slots are contiguous in one
    partition.  A hardware batched gather (InstDMAGatherAnt, 128B
    payload / 256B stride descriptors) fetches embeds[col] for every
    slot; pad slots point at an all-zero table row.
  - DVE scales by vals (broadcast multiply) and does a static
    segmented reduction (tensor_reduce over the K4 axis), accumulating
    the 4 col-chunks into a resident [128, 98, 32] output tile.
  - Rows with more than K4 edges in a chunk spill to small per-128-row
    "overflow" batches handled with indirect gather + CCE-add scatter.
  - Dense output write, host concatenates the 8 row blocks.
"""

import sys

if "/opt/trn_rl_repo" not in sys.path:
    sys.path.insert(0, "/opt/trn_rl_repo")

import numpy as np

import concourse.bass as bass
import concourse.tile as tile
from concourse import bacc, mybir, bass_utils
from concourse import ap_utils
from concourse.bass import round_up_to_multiple, exact_div
from concourse.tile import add_dep_helper

# ---------------- problem geometry (hardcoded) ----------------
N_NODES = 100000
N_EDGES = 3200000
D = 32
NC = 8
RPC = N_NODES // NC            # rows per core = 12500
RPAD = 12544                   # rows padded to multiple of 128 (98 * 128)
QROWS = RPAD // 128            # 98 free "q" rows per partition
CHUNK = 25000                  # nodes per table chunk (balanced, int16-safe)
NCHUNKS = 4
K4 = 12                        # slots per (row, chunk)
COLS = QROWS * K4              # free columns per chunk's slot block = 1176
SLOTS = COLS * 128             # slots per chunk = 150528
TCOL = 24                      # free columns per gather instruction
NBLK = COLS // TCOL            # 49 gather instructions per chunk
NIDX = TCOL * 128              # 3072 indices per gather instruction
ELEM = 32                      # gathered payload elems (128B)
STEP = 64                      # table row stride elems (256B)
PAD_LIDX = CHUNK               # local index of the zero row in each chunk
DEAD_ROW = RPC + 10            # scatter target for overflow padding lanes

_cache = {}


def _dma_gather_raw(gp, out_ap, in_ap, idxs_ap, num_idxs, num_idxs_reg,
                    elem_size, elem_step, queue_num=0):
    """dma_gather with a 128-byte payload on a 256-byte-strided table.

    Identical to BassGpSimd.dma_gather(transpose=False) minus the
    blanket elem_size_bytes%256 assert (only the descriptor stride needs
    the 256B granularity; payload size is free).  Validated on HW.
    """
    assert idxs_ap.dtype == mybir.dt.int16
    assert in_ap.dtype == out_ap.dtype
    assert in_ap.space == bass.MemorySpace.DRAM
    assert idxs_ap.space == bass.MemorySpace.SBUF
    assert out_ap.space == bass.MemorySpace.SBUF
    assert ap_utils.ap_is_contiguous(in_ap.ap[1:])
    assert ap_utils.ap_is_contiguous(out_ap.ap[1:])
    assert ap_utils.ap_is_contiguous(idxs_ap.ap[1:])
    assert in_ap.ap[-1][1] == out_ap.ap[-1][1] == elem_size
    assert out_ap.ap[0][1] * out_ap.ap[1][1] == round_up_to_multiple(num_idxs, 128)
    assert in_ap.ap[0][0] == elem_step
    stride_bytes = elem_step * mybir.dt.size(in_ap.dtype)
    stride_bytes_256 = exact_div(stride_bytes, 256)
    assert stride_bytes_256 < 256
    _in_ap = gp.lower_ap_dma(in_ap, for_custom_bir_dma=True)
    _idxs_ap = gp.lower_ap(idxs_ap)
    _out_ap = gp.lower_ap(out_ap)
    return gp.add_instruction(
        mybir.InstDMAGatherAnt(
            name=gp.bass.get_next_instruction_name(),
            ins=[*_in_ap, _idxs_ap, gp.lower_val_access(gp.to_reg(num_idxs_reg))],
            outs=[_out_ap],
            transpose=False,
            num_idxs=num_idxs,
            elem_size=elem_size,
            stride_bytes_256=stride_bytes_256,
            gen_mode=0,
            single_packet=False,
            queue_num=queue_num,
            sbuf_tokens_per_rank=0,
            sbuf_free_dim_per_rank=0,
            sbuf_free_dim_pad_per_rank=0,
            sbuf_byte_offset=0,
        )
    )


def _build_module(novi):
    """Build + finalize the per-core SPMD module. novi = number of
    128-lane overflow batches (identical across cores)."""
    nc = bacc.Bacc("TRN2", target_bir_lowering=False, num_swdge_queues=4)
    f32, i16, i32 = mybir.dt.float32, mybir.dt.int16, mybir.dt.int32

    tabp = nc.dram_tensor("tabp", [NCHUNKS * 32768, STEP], f32, kind="ExternalInput")
    emb = nc.dram_tensor("emb", [N_NODES, D], f32, kind="ExternalInput")
    idxs = nc.dram_tensor("idxs", [NCHUNKS, 128, SLOTS // 16], i16, kind="ExternalInput")
    vals = nc.dram_tensor("vals", [NCHUNKS, 128, COLS], f32, kind="ExternalInput")
    if novi:
        oidx = nc.dram_tensor("oidx", [novi, 128, 1], i32, kind="ExternalInput")
        oval = nc.dram_tensor("oval", [novi, 128, 1], f32, kind="ExternalInput")
        orow = nc.dram_tensor("orow", [novi, 128, 1], i32, kind="ExternalInput")
    y = nc.dram_tensor("y", [RPAD, D], f32, kind="ExternalOutput")

    with tile.TileContext(nc) as tc:
        with tc.tile_pool(name="acc", bufs=1) as accp, \
             tc.tile_pool(name="work", bufs=6) as wp:
            out_acc = accp.tile([128, QROWS * D], f32)
            nc.vector.memset(out_acc[:], 0.0)

            for c in range(NCHUNKS):
                tab_c = tabp[c * 32768:(c + 1) * 32768, :ELEM]
                for b in range(NBLK):
                    idx_t = wp.tile([128, TCOL * 8], i16, tag="idx")
                    val_t = wp.tile([128, TCOL], f32, tag="val")
                    g_t = wp.tile([128, TCOL, ELEM], f32, tag="g")
                    red_t = wp.tile([128, (TCOL // K4) * D], f32, tag="red")
                    nc.sync.dma_start(
                        out=idx_t[:],
                        in_=idxs[c, :, b * TCOL * 8:(b + 1) * TCOL * 8])
                    nc.scalar.dma_start(
                        out=val_t[:],
                        in_=vals[c, :, b * TCOL:(b + 1) * TCOL])
                    _dma_gather_raw(
                        nc.gpsimd, g_t[:], tab_c, idx_t[:],
                        num_idxs=NIDX, num_idxs_reg=NIDX,
                        elem_size=ELEM, elem_step=STEP,
                        queue_num=b % 4)
                    # scale by vals (broadcast along d)
                    vb = val_t[:].unsqueeze(-1).broadcast_to((128, TCOL, ELEM))
                    nc.vector.tensor_tensor(
                        out=g_t[:], in0=g_t[:], in1=vb,
                        op=mybir.AluOpType.mult)
                    # segmented reduce over the K4 axis (innermost, strided)
                    rin = g_t[:].rearrange("p (q k) d -> p q d k", k=K4)
                    nc.vector.tensor_reduce(
                        out=red_t[:].rearrange("p (q d) -> p q d", d=D),
                        in_=rin, axis=mybir.AxisListType.X,
                        op=mybir.AluOpType.add)
                    q0 = b * (TCOL // K4)
                    q1 = q0 + TCOL // K4
                    nc.vector.tensor_tensor(
                        out=out_acc[:, q0 * D:q1 * D],
                        in0=out_acc[:, q0 * D:q1 * D],
                        in1=red_t[:],
                        op=mybir.AluOpType.add)

            dense_wr = nc.sync.dma_start(
                out=y[:, :].rearrange("(q p) d -> p q d", p=128),
                in_=out_acc[:].rearrange("p (q d) -> p q d", d=D))

            prev = dense_wr
            for j in range(novi):
                oi_t = wp.tile([128, 1], i32, tag="oi")
                ov_t = wp.tile([128, 1], f32, tag="ov")
                or_t = wp.tile([128, 1], i32, tag="or")
                og_t = wp.tile([128, D], f32, tag="og")
                nc.sync.dma_start(out=oi_t[:], in_=oidx[j, :, :])
                nc.scalar.dma_start(out=ov_t[:], in_=oval[j, :, :])
                nc.sync.dma_start(out=or_t[:], in_=orow[j, :, :])
                nc.gpsimd.indirect_dma_start(
                    out=og_t[:], out_offset=None,
                    in_=emb[:, :],
                    in_offset=bass.IndirectOffsetOnAxis(ap=oi_t[:, :1], axis=0))
                nc.vector.tensor_scalar(
                    out=og_t[:], in0=og_t[:],
                    scalar1=ov_t[:, :1], scalar2=None,
                    op0=mybir.AluOpType.mult)
                sc = nc.gpsimd.indirect_dma_start(
                    out=y[:, :],
                    out_offset=bass.IndirectOffsetOnAxis(ap=or_t[:, :1], axis=0),
                    in_=og_t[:], in_offset=None,
                    compute_op=mybir.AluOpType.add)
                # CCE-add RMWs on y must run after the dense write and
                # serialize among themselves (rows can repeat across
                # batches); Tile's shadow-memory tracking orders same-
                # tensor DRAM accesses, verified by the row check in test.py
                prev = sc

    nc.finalize()
    return nc


def _preprocess(adj_row, adj_col, adj_vals, embeds):
    """Host-side shard + layout. Returns (in_maps, novi)."""
    order = np.argsort(adj_row, kind="stable")
    rows = np.ascontiguousarray(adj_row[order])
    cols = np.ascontiguousarray(adj_col[order])
    vals = np.ascontiguousarray(adj_vals[order]).astype(np.float32)

    # replicated padded table: chunk c rows [c*32768, c*32768+32767) hold
    # nodes [c*CHUNK, c*CHUNK+32767); row c*32768+32767 stays zero (pad)
    tabp = np.zeros((NCHUNKS * 32768, STEP), dtype=np.float32)
    for c in range(NCHUNKS):
        lo = c * CHUNK
        hi = min(lo + CHUNK, N_NODES)
        if lo < N_NODES:
            tabp[c * 32768:c * 32768 + (hi - lo), :D] = embeds[lo:hi]

    bounds = np.searchsorted(rows, np.arange(NC + 1) * RPC)

    per_core = []
    max_novi = 0
    for m in range(NC):
        s, e = bounds[m], bounds[m + 1]
        rl = (rows[s:e] - m * RPC).astype(np.int64)
        cc = cols[s:e].astype(np.int64)
        vv = vals[s:e]
        ch = cc // CHUNK
        lidx = cc % CHUNK

        # pad slots carry val=0, so they may gather ANY row (junk*0 == 0).
        # Spread them over the chunk so they don't hammer one HBM row
        # (a constant index measured 2.5x slower than random access).
        rng = np.random.default_rng(12345 + m)
        idx_arr = rng.integers(0, CHUNK, size=(NCHUNKS, SLOTS)).astype(np.int16)
        val_arr = np.zeros((NCHUNKS, SLOTS), dtype=np.float32)
        over = []  # (global_col, val, row_local, excess_rank)
        for c in range(NCHUNKS):
            msk = ch == c
            crl = rl[msk]          # sorted (rows sorted, mask keeps order)
            cli = lidx[msk]
            cvv = vv[msk]
            if crl.size == 0:
                continue
            uniq, starts, counts = np.unique(
                crl, return_index=True, return_counts=True)
            k = np.arange(crl.size) - np.repeat(starts, counts)
            main = k < K4
            mrl, mk = crl[main], k[main]
            slot = ((mrl // 128) * K4 + mk) * 128 + (mrl % 128)
            idx_arr[c, slot] = cli[main].astype(np.int16)
            val_arr[c, slot] = cvv[main]
            ov = ~main
            if ov.any():
                over.append((cc[msk][ov], cvv[ov], crl[ov], k[ov] - K4))

        # wrapped idx layout [128, SLOTS//16] (j%16 partition, replicated x8)
        idx_w = np.tile(
            idx_arr.reshape(NCHUNKS, SLOTS // 16, 16).transpose(0, 2, 1),
            (1, 8, 1))
        val_w = val_arr.reshape(NCHUNKS, COLS, 128).transpose(0, 2, 1)
        val_w = np.ascontiguousarray(val_w)

        # overflow batches: excess-rank groups chunked into 128-lane
        # batches; rows within a batch are distinct by construction
        ob = []
        if over:
            ocol = np.concatenate([o[0] for o in over])
            ovv = np.concatenate([o[1] for o in over])
            orl = np.concatenate([o[2] for o in over])
            # recompute excess rank ACROSS chunks per row for distinctness
            o_order = np.argsort(orl, kind="stable")
            ocol, ovv, orl = ocol[o_order], ovv[o_order], orl[o_order]
            uniq, starts, counts = np.unique(
                orl, return_index=True, return_counts=True)
            be = np.arange(orl.size) - np.repeat(starts, counts)
            for b in range(int(be.max()) + 1):
                sel = be == b
                sc_, sv_, sr_ = ocol[sel], ovv[sel], orl[sel]
                for i in range(0, sc_.size, 128):
                    ci = np.zeros(128, np.int32)
                    cv = np.zeros(128, np.float32)
                    cr = np.full(128, DEAD_ROW, np.int32)
                    n = min(128, sc_.size - i)
                    ci[:n] = sc_[i:i + n]
                    cv[:n] = sv_[i:i + n]
                    cr[:n] = sr_[i:i + n]
                    ob.append((ci, cv, cr))
        max_novi = max(max_novi, len(ob))
        per_core.append((idx_w, val_w, ob))

    novi = max_novi
    in_maps = []
    for m in range(NC):
        idx_w, val_w, ob = per_core[m]
        im = {
            "tabp": tabp,
            "emb": np.ascontiguousarray(embeds.astype(np.float32)),
            "idxs": np.ascontiguousarray(idx_w),
            "vals": val_w,
        }
        if novi:
            oidx = np.zeros((novi, 128, 1), np.int32)
            oval = np.zeros((novi, 128, 1), np.float32)
            orow = np.full((novi, 128, 1), DEAD_ROW, np.int32)
            for j, (ci, cv, cr) in enumerate(ob):
                oidx[j, :, 0] = ci
                oval[j, :, 0] = cv
                orow[j, :, 0] = cr
            im["oidx"], im["oval"], im["orow"] = oidx, oval, orow
        in_maps.append(im)
    return in_maps, novi


def _run(in_maps, novi, trace=False):
    key = ("mod", novi)
    if key not in _cache:
        _cache[key] = _build_module(novi)
    nc = _cache[key]
    return bass_utils.run_bass_kernel_spmd(
        nc, in_maps, core_ids=list(range(NC)), trace=trace)


def kernel(adj_row, adj_col, adj_vals, embeds, _trace=False, _return_res=False):
    adj_row = np.asarray(adj_row)
    adj_col = np.asarray(adj_col)
    adj_vals = np.asarray(adj_vals)
    embeds = np.asarray(embeds)
    in_maps, novi = _preprocess(adj_row, adj_col, adj_vals, embeds)
    res = _run(in_maps, novi, trace=_trace)
    out = np.concatenate(
        [res.results[m]["y"][:RPC] for m in range(NC)], axis=0)
    out = np.ascontiguousarray(out, dtype=np.float32)
    if _return_res:
        return out, res
    return out

